# revision 8
# baseline (speedup 1.0000x reference)
"""AAGA (agent attention) Trainium2 kernel, data-parallel over batch B=8 on 8 NeuronCores.

Math (per batch b, d=256, K=64 agents, N=8192 tokens), with all weight-only
products folded on the host:
  s1 = (q_agent @ Wk) @ x.T * sc + (q_agent.b_k)*sc            # [K, N]
  attn1 = softmax(s1, over N)       (no max-subtract: |logits| ~ 1)
  v1f2 = ((attn1 @ x) @ Wv.T + b_v) @ Wfc1.T + b_fc1) @ Wfc2.T + b_fc2
  s2 = x @ (k_agent @ Wq).T * sc + (k_agent.b_q)*sc            # [N, K]
  out = rmsnorm(softmax(s2, over K) @ v1f2 + x, norm_scale)

Device layouts: x resident token-major f32 (exact residual) + bf16 copy +
feature-major bf16 (built by SB->SB DMA-transpose). Matmuls in bf16
(PSUM accumulation f32); softmax normalizers via ACT accum_out.
"""

import os
import sys

import numpy as np

for _p in ("/opt/trn_rl_repo", "/opt/pypackages"):
    if os.path.isdir(_p) and _p not in sys.path:
        sys.path.append(_p)

import ml_dtypes

import concourse.bass as bass
import concourse.bacc as bacc
import concourse.mybir as mybir
import concourse.tile as tile
from concourse.bass_utils import run_bass_kernel_spmd

F32 = mybir.dt.float32
B16 = mybir.dt.bfloat16
AF = mybir.ActivationFunctionType
ALU = mybir.AluOpType
AX = mybir.AxisListType

B, N, D, K = 8, 8192, 256, 64
NT = N // 128       # 64 token tiles of 128
NSLAB = N // 512    # 16 slabs of 512 tokens
SC = float(D) ** -0.5
EPS = 1e-8

_CACHE = {}


def _build(apply_bqk: bool, apply_ns: bool) -> bass.Bass:
    nc = bacc.Bacc("TRN2", target_bir_lowering=False, debug=False, num_devices=8)

    x_ext = nc.declare_dram_parameter("x", [N, D], F32, isOutput=False)
    qk_d = nc.declare_dram_parameter("qk_eff_t", [128, 2, K], B16, isOutput=False)
    wq_d = nc.declare_dram_parameter("wq_eff_t", [128, 2, K], B16, isOutput=False)
    s1b_d = nc.declare_dram_parameter("s1_bias", [K, 1], F32, isOutput=False)
    wv_d = nc.declare_dram_parameter("wv_t", [128, 2, D], B16, isOutput=False)
    wf1_d = nc.declare_dram_parameter("wfc1_t", [128, 2, D], B16, isOutput=False)
    wf2_d = nc.declare_dram_parameter("wfc2_t", [128, 2, D], B16, isOutput=False)
    br_d = nc.declare_dram_parameter("b_rows", [1, 3 * D], B16, isOutput=False)
    idf_d = nc.declare_dram_parameter("ident_f32", [128, 128], F32, isOutput=False)
    idb_d = nc.declare_dram_parameter("ident_b16", [128, 128], B16, isOutput=False)
    on_d = nc.declare_dram_parameter("ones_col", [1, K], B16, isOutput=False)
    bqk_d = nc.declare_dram_parameter("bqk_bcast", [128, K], F32, isOutput=False)
    ns_d = nc.declare_dram_parameter("ns_bcast", [128, D], F32, isOutput=False)
    out_ext = nc.declare_dram_parameter("out", [N, D], F32, isOutput=True)

    with tile.TileContext(nc, num_cores=8) as tc:
        with tc.tile_pool(name="const", bufs=1) as cpool:
            qk_sb = cpool.tile([128, 2, K], B16)
            nc.sync.dma_start(out=qk_sb[:], in_=qk_d[:])
            wq_sb = cpool.tile([128, 2, K], B16)
            nc.sync.dma_start(out=wq_sb[:], in_=wq_d[:])
            s1b_sb = cpool.tile([K, 1], F32)
            nc.sync.dma_start(out=s1b_sb[:], in_=s1b_d[:])
            wv_sb = cpool.tile([128, 2, D], B16)
            nc.sync.dma_start(out=wv_sb[:], in_=wv_d[:])
            wf1_sb = cpool.tile([128, 2, D], B16)
            nc.sync.dma_start(out=wf1_sb[:], in_=wf1_d[:])
            wf2_sb = cpool.tile([128, 2, D], B16)
            nc.sync.dma_start(out=wf2_sb[:], in_=wf2_d[:])
            br_sb = cpool.tile([1, 3 * D], B16)
            nc.sync.dma_start(out=br_sb[:], in_=br_d[:])
            idf_sb = cpool.tile([128, 128], F32)
            nc.sync.dma_start(out=idf_sb[:], in_=idf_d[:])
            idb_sb = cpool.tile([128, 128], B16)
            nc.sync.dma_start(out=idb_sb[:], in_=idb_d[:])
            on_sb = cpool.tile([1, K], B16)
            nc.sync.dma_start(out=on_sb[:], in_=on_d[:])
            bqk_sb = cpool.tile([128, K], F32)
            nc.sync.dma_start(out=bqk_sb[:], in_=bqk_d[:])
            ns_sb = cpool.tile([128, D], F32)
            nc.sync.dma_start(out=ns_sb[:], in_=ns_d[:])

            # residents
            xsb = cpool.tile([128, NT, D], F32)      # token-major x (exact)
            xsb_b = cpool.tile([128, NT, D], B16)    # token-major x, bf16
            xT0 = cpool.tile([128, N], B16)          # x.T rows 0..127 (bf16)
            xT1 = cpool.tile([128, N], B16)          # x.T rows 128..255
            z1p = cpool.tile([K, NSLAB], F32)        # per-slab stage-1 sum(exp)
            v1f2 = cpool.tile([K, D], B16)           # final per-agent values

            # ---------------- Phase A ----------------
            with tc.tile_pool(name="pA_xp", bufs=2, space="PSUM") as ps_x, \
                 tc.tile_pool(name="pA_s1", bufs=2, space="PSUM") as ps_s1, \
                 tc.tile_pool(name="pA_e1T", bufs=2, space="PSUM") as ps_e1t, \
                 tc.tile_pool(name="pA_u1", bufs=1, space="PSUM") as ps_u1, \
                 tc.tile_pool(name="pA_sb", bufs=3) as pa:
                u1 = ps_u1.tile([K, D], F32, name="u1")
                for slab in range(NSLAB):
                    for j in range(4):
                        t = slab * 4 + j
                        nc.sync.dma_start(
                            out=xsb[:, t, :], in_=x_ext[t * 128:(t + 1) * 128, :]
                        )
                        nc.scalar.activation(
                            xsb_b[:, t, :], xsb[:, t, :], AF.Copy
                        )
                        for c in range(2):
                            xp = ps_x.tile([128, 128], F32, name="xp", tag="xp")
                            nc.tensor.transpose(
                                xp[:], xsb[:, t, c * 128:(c + 1) * 128], idf_sb[:]
                            )
                            xTc = xT0 if c == 0 else xT1
                            nc.vector.tensor_copy(
                                out=xTc[:, t * 128:(t + 1) * 128], in_=xp[:]
                            )
                    s1 = ps_s1.tile([K, 512], F32, name="s1", tag="s1")
                    for c in range(2):
                        xTc = xT0 if c == 0 else xT1
                        nc.tensor.matmul(
                            s1[:],
                            qk_sb[:, c, :],
                            xTc[:, slab * 512:(slab + 1) * 512],
                            start=(c == 0),
                            stop=(c == 1),
                        )
                    e1 = pa.tile([K, 512], F32, name="e1", tag="e1")
                    nc.scalar.activation(
                        e1[:], s1[:], AF.Exp,
                        bias=s1b_sb[:], scale=SC,
                        accum_out=z1p[:, slab:slab + 1],
                    )
                    for j in range(4):
                        t = slab * 4 + j
                        ep = ps_e1t.tile([128, K], F32, name="ep", tag="ep")
                        nc.tensor.transpose(
                            ep[:], e1[:, j * 128:(j + 1) * 128], idf_sb[:K, :K]
                        )
                        eT = pa.tile([128, K], B16, name="eT", tag="eT")
                        nc.scalar.activation(eT[:], ep[:], AF.Identity)
                        nc.tensor.matmul(
                            u1[:],
                            eT[:],
                            xsb_b[:, t, :],
                            start=(t == 0),
                            stop=(t == NT - 1),
                        )

            # ---------------- Phase B ----------------
            with tc.tile_pool(name="pB_ps", bufs=2, space="PSUM") as psb, \
                 tc.tile_pool(name="pB_sb", bufs=2) as pb:
                z1 = pb.tile([K, 1], F32, name="z1")
                nc.vector.tensor_reduce(z1[:], z1p[:], axis=AX.X, op=ALU.add)
                r1 = pb.tile([K, 1], F32, name="r1")
                nc.vector.reciprocal(r1[:], z1[:])
                cur = pb.tile([K, D], F32, name="chain0")
                nc.scalar.activation(cur[:], u1[:], AF.Identity, scale=r1[:])
                for wi, (w_sb, boff) in enumerate(
                    [(wv_sb, 0), (wf1_sb, D), (wf2_sb, 2 * D)]
                ):
                    vT = pb.tile([128, 2, K], B16, name=f"vT{wi}", tag="vT")
                    for c in range(2):
                        tp = psb.tile([128, K], F32, name=f"tp{wi}{c}", tag="tp")
                        nc.tensor.transpose(
                            tp[:], cur[:, c * 128:(c + 1) * 128], idf_sb[:K, :K]
                        )
                        nc.scalar.activation(vT[:, c, :], tp[:], AF.Identity)
                    nxt = psb.tile([K, D], F32, name=f"ch{wi}", tag="chps")
                    for c in range(2):
                        nc.tensor.matmul(
                            nxt[:],
                            vT[:, c, :],
                            w_sb[:, c, :],
                            start=(c == 0),
                            stop=False,
                        )
                    nc.tensor.matmul(
                        nxt[:], on_sb[:], br_sb[:, boff:boff + D],
                        start=False, stop=True,
                    )
                    if wi == 2:
                        nc.scalar.activation(v1f2[:], nxt[:], AF.Identity)
                    else:
                        dst = pb.tile(
                            [K, D], F32, name=f"chain{wi + 1}", tag=f"chain{wi + 1}"
                        )
                        nc.scalar.activation(dst[:], nxt[:], AF.Identity)
                        cur = dst

            # ---------------- Phase C ----------------
            with tc.tile_pool(name="pC_s2", bufs=2, space="PSUM") as ps_s2, \
                 tc.tile_pool(name="pC_e2T", bufs=2, space="PSUM") as ps_e2t, \
                 tc.tile_pool(name="pC_v2", bufs=2, space="PSUM") as ps_v2, \
                 tc.tile_pool(name="pC_sb", bufs=3) as pc:
                for t in range(NT):
                    s2 = ps_s2.tile([128, K], F32, name="s2", tag="s2")
                    for c in range(2):
                        xTc = xT0 if c == 0 else xT1
                        nc.tensor.matmul(
                            s2[:],
                            xTc[:, t * 128:(t + 1) * 128],
                            wq_sb[:, c, :],
                            start=(c == 0),
                            stop=(c == 1),
                        )
                    e2 = pc.tile([128, K], B16, name="e2", tag="e2")
                    z2 = pc.tile([128, 1], F32, name="z2", tag="z2")
                    if apply_bqk:
                        sraw = pc.tile([128, K], F32, name="sraw", tag="sraw")
                        nc.vector.tensor_tensor(
                            out=sraw[:], in0=s2[:], in1=bqk_sb[:], op=ALU.add
                        )
                        esrc = sraw
                    else:
                        esrc = s2
                    nc.scalar.activation(
                        e2[:], esrc[:], AF.Exp, scale=SC, accum_out=z2[:]
                    )
                    r2 = pc.tile([128, 1], F32, name="r2", tag="r2")
                    nc.vector.reciprocal(r2[:], z2[:])
                    e2n = pc.tile([128, K], F32, name="e2n", tag="e2n")
                    nc.scalar.activation(e2n[:], e2[:], AF.Identity, scale=r2[:])
                    eTp = ps_e2t.tile([K, 128], F32, name="eTp", tag="eTp")
                    nc.tensor.transpose(eTp[:], e2n[:], idf_sb[:])
                    e2T = pc.tile([K, 128], B16, name="e2T", tag="e2T")
                    nc.scalar.activation(e2T[:], eTp[:], AF.Identity)
                    v2 = ps_v2.tile([128, D], F32, name="v2", tag="v2")
                    nc.tensor.matmul(v2[:], e2T[:], v1f2[:], start=True, stop=True)
                    y = pc.tile([128, D], F32, name="y", tag="y")
                    nc.vector.tensor_tensor(
                        out=y[:], in0=v2[:], in1=xsb[:, t, :], op=ALU.add
                    )
                    q2 = pc.tile([128, D], B16, name="q2", tag="q2")
                    ms = pc.tile([128, 1], F32, name="ms", tag="ms")
                    # Square(y/16) = y^2/256; accum_out -> ms = sum(y^2)/256
                    nc.scalar.activation(
                        q2[:], y[:], AF.Square, scale=1.0 / 16.0, accum_out=ms[:]
                    )
                    r = pc.tile([128, 1], F32, name="r", tag="r")
                    nc.scalar.activation(r[:], ms[:], AF.Sqrt)
                    rr = pc.tile([128, 1], F32, name="rr", tag="rr")
                    nc.vector.tensor_scalar_add(rr[:], r[:], EPS)
                    r3 = pc.tile([128, 1], F32, name="r3", tag="r3")
                    nc.vector.reciprocal(r3[:], rr[:])
                    ot = pc.tile([128, D], F32, name="ot", tag="ot")
                    nc.scalar.activation(ot[:], y[:], AF.Identity, scale=r3[:])
                    if apply_ns:
                        ot2 = pc.tile([128, D], F32, name="ot2", tag="ot2")
                        nc.vector.tensor_tensor(
                            out=ot2[:], in0=ot[:], in1=ns_sb[:], op=ALU.mult
                        )
                        ot = ot2
                    nc.sync.dma_start(
                        out=out_ext[t * 128:(t + 1) * 128, :], in_=ot[:]
                    )
    nc.compile()
    return nc


def _make_runner(nc):
    """Build a cached jitted executor for nc (mirrors bass2jax.run_bass_via_pjrt
    multi-core path, but reusable across calls)."""
    import jax
    import numpy as _np
    from jax.sharding import Mesh, PartitionSpec
    from jax.experimental.shard_map import shard_map
    from concourse import bass2jax as b2j

    b2j.install_neuronx_cc_hook()

    partition_name = nc.partition_id_tensor.name if nc.partition_id_tensor else None
    in_names, out_names, out_avals, zero_shapes = [], [], [], []
    for alloc in nc.m.functions[0].allocations:
        if not isinstance(alloc, mybir.MemoryLocationSet):
            continue
        name = alloc.memorylocations[0].name
        if alloc.kind == "ExternalInput":
            if name != partition_name:
                in_names.append(name)
        elif alloc.kind == "ExternalOutput":
            out_names.append(name)
            shape = tuple(alloc.tensor_shape)
            dtype = mybir.dt.np(alloc.dtype)
            out_avals.append(jax.core.ShapedArray(shape, dtype))
            zero_shapes.append((shape, dtype))
    n_params, n_outs = len(in_names), len(out_avals)
    all_in_names = list(in_names) + list(out_names)
    if partition_name is not None:
        all_in_names.append(partition_name)
    donate = tuple(range(n_params, n_params + n_outs))

    def _body(*args):
        operands = list(args)
        if partition_name is not None:
            operands.append(b2j.partition_id_tensor())
        outs = b2j._bass_exec_p.bind(
            *operands,
            out_avals=tuple(out_avals),
            in_names=tuple(all_in_names),
            out_names=tuple(out_names),
            lowering_input_output_aliases=(),
            sim_require_finite=True,
            sim_require_nnan=True,
            nc=nc,
        )
        return tuple(outs)

    devices = jax.devices()[:B]
    mesh = Mesh(_np.asarray(devices), ("core",))
    in_specs = (PartitionSpec("core"),) * (n_params + n_outs)
    out_specs = (PartitionSpec("core"),) * n_outs
    sharded = jax.jit(
        shard_map(_body, mesh=mesh, in_specs=in_specs, out_specs=out_specs,
                  check_rep=False),
        donate_argnums=donate,
        keep_unused=True,
    )

    def run(in_maps):
        per_core = [[_np.asarray(m[name]) for name in in_names] for m in in_maps]
        concat_in = [
            _np.concatenate([per_core[c][i] for c in range(B)], axis=0)
            for i in range(n_params)
        ]
        concat_zeros = [
            _np.zeros((B * sh[0], *sh[1:]), dt) for (sh, dt) in zero_shapes
        ]
        out_arrs = sharded(*concat_in, *concat_zeros)
        return [
            {
                name: _np.asarray(out_arrs[i]).reshape(B, *out_avals[i].shape)[c]
                for i, name in enumerate(out_names)
            }
            for c in range(B)
        ]

    run.sharded = sharded
    run.in_names = in_names
    run.zero_shapes = zero_shapes
    run.out_names = out_names
    run.out_avals = out_avals
    return run


def _get_runner(apply_bqk: bool, apply_ns: bool):
    key = (apply_bqk, apply_ns)
    if key not in _CACHE:
        nc = _build(apply_bqk, apply_ns)
        _CACHE[key] = _make_runner(nc)
    return _CACHE[key]


def _bf16(a):
    return np.ascontiguousarray(a.astype(ml_dtypes.bfloat16))


def kernel(agent, x, W_qkv, b_qkv, W_agent, b_agent, W_fc1, b_fc1, W_fc2, b_fc2,
           norm_scale):
    agent = np.asarray(agent, dtype=np.float32)
    x = np.asarray(x, dtype=np.float32)
    W_qkv = np.asarray(W_qkv, dtype=np.float32)
    b_qkv = np.asarray(b_qkv, dtype=np.float32)
    W_agent = np.asarray(W_agent, dtype=np.float32)
    b_agent = np.asarray(b_agent, dtype=np.float32)
    W_fc1 = np.asarray(W_fc1, dtype=np.float32)
    b_fc1 = np.asarray(b_fc1, dtype=np.float32)
    W_fc2 = np.asarray(W_fc2, dtype=np.float32)
    b_fc2 = np.asarray(b_fc2, dtype=np.float32)
    norm_scale = np.asarray(norm_scale, dtype=np.float32)

    # host-side weight folding (all O(d^2))
    qa_ka = agent @ W_agent.T + b_agent
    q_agent, k_agent = qa_ka[:, :D], qa_ka[:, D:]
    Wq, Wk, Wv = W_qkv[:D], W_qkv[D:2 * D], W_qkv[2 * D:]
    b_q, b_k, b_v = b_qkv[:D], b_qkv[D:2 * D], b_qkv[2 * D:]

    qk_eff = q_agent @ Wk                      # [K, D]
    s1_bias = ((q_agent @ b_k) * SC).reshape(K, 1)
    wq_eff = k_agent @ Wq                      # [K, D]
    bqk = k_agent @ b_q                        # [K]
    apply_bqk = bool(np.any(bqk != 0.0))
    apply_ns = not bool(np.allclose(norm_scale, 1.0))

    def dmaj(m):  # [K, D] -> [128, 2, K]  (feature-major chunks for lhsT)
        return m.T.reshape(2, 128, -1).transpose(1, 0, 2)

    def rmaj(w):  # [dout, din] -> [128, 2, dout]  (w.T chunks for rhs)
        return w.T.reshape(2, 128, -1).transpose(1, 0, 2)

    shared = {
        "qk_eff_t": _bf16(dmaj(qk_eff)),
        "wq_eff_t": _bf16(dmaj(wq_eff)),
        "s1_bias": np.ascontiguousarray(s1_bias, dtype=np.float32),
        "wv_t": _bf16(rmaj(Wv)),
        "wfc1_t": _bf16(rmaj(W_fc1)),
        "wfc2_t": _bf16(rmaj(W_fc2)),
        "b_rows": _bf16(np.concatenate([b_v, b_fc1, b_fc2]).reshape(1, 3 * D)),
        "ident_f32": np.eye(128, dtype=np.float32),
        "ident_b16": _bf16(np.eye(128, dtype=np.float32)),
        "ones_col": _bf16(np.ones((1, K), dtype=np.float32)),
        "bqk_bcast": np.ascontiguousarray(
            np.tile(bqk.reshape(1, K), (128, 1)), dtype=np.float32
        ),
        "ns_bcast": np.ascontiguousarray(
            np.tile(norm_scale.reshape(1, D), (128, 1)), dtype=np.float32
        ),
    }

    in_maps = [dict(shared, x=np.ascontiguousarray(x[b])) for b in range(B)]

    runner = _get_runner(apply_bqk, apply_ns)
    results = runner(in_maps)
    out = np.stack([results[b]["out"] for b in range(B)], axis=0)
    return out.astype(np.float32)
